# revision 1
# baseline (speedup 1.0000x reference)
"""Trainium2 Bass kernel for nn_L2GESRModule.

Reference computation:
    Fh_conv = Fh @ Wh + bh            (dead: only used via ones_like)
    ESF     = ones_like(Fh_conv)      -> gather indices are a fixed shift
    Y       = Fl @ Wl + bl
    out[b,i,j,:] = Y[b, min(i+1,H-1), min(j+1,W-1), :]

The whole problem is one 1x1-conv GEMM on Fl plus a static (+1,+1)
clamped-shift, data-parallel over batch (1 batch element per core). The
Fh/Wh/bh branch contributes nothing and is never loaded.

Flat-pixel layout: image = 16384 pixels; out[O] = Y[O + 129] except
col-127 cells (O%128==127) which need Y[O + 128] (clamped col), and the
last row which duplicates row H-2.

Chunks of CH=128*GK pixels: SBUF tiles [128 parts, GK slots, 256 ch],
partition p = GK *consecutive* pixels -> GK KB contiguous per partition ->
128 large DMA descriptors per transfer (HWDGE descriptor-generation is the
bottleneck with small descriptors). Uniform chunk c loads src window
[CH*c+129, +CH) so every compute group k writes ybig[:, k] unshifted.
Col-127 cells then duplicate the col-126 value (previous slot, on
partitions p % (128//GK) == 128//GK - 1): engines cannot address strided
partitions, so the patch is a masked copy_predicated. The last chunk's
window would run off the input, so it loads [P-CH+128, P) (+128-style),
shifting group 0's result by one partition via a small SBUF->SBUF DMA.

Compute per 128-pixel group: 2x PE transpose (fp32) -> PSUM -> ACT evac to
SBUF as X^T (cast to fp32r) -> 2x PE matmul (fp32r, full rate at N=256)
accumulate in PSUM -> DVE adds bias PSUM->SBUF.

Loads go out on the SP HWDGE ring (nc.sync), stores on the ACT HWDGE ring
(nc.scalar) so both physical descriptor rings / all 16 SDMA engines run.
Aggregate traffic (~34MB/core) sits at the ~358 GB/s HBM-per-core limit.
"""

import numpy as np

import concourse.bacc as bacc
import concourse.mybir as mybir
from concourse import bass_utils, tile
from concourse.masks import make_identity

B, H, W, CIN, COUT = 8, 128, 128, 256, 256
N_CORES = 8
MM_DT = mybir.dt.float32r  # fp32r: full-rate PE, ~19-bit mantissa products
GK = 16                    # pixel-slots per partition per chunk


def build_nc(n_rows: int = H, mm_dt=MM_DT):
    f32 = mybir.dt.float32
    P = n_rows * W  # total pixels per image
    CH = 128 * GK   # pixels per chunk
    assert P % CH == 0 and P >= CH
    assert 128 % GK == 0
    n_chunks = P // CH

    nc = bacc.Bacc("TRN2", target_bir_lowering=False, debug=False)
    Fl = nc.dram_tensor("Fl", [P, CIN], f32, kind="ExternalInput").ap()
    Wl = nc.dram_tensor("Wl", [CIN, COUT], f32, kind="ExternalInput").ap()
    bl = nc.dram_tensor("bl", [COUT], f32, kind="ExternalInput").ap()
    # mask over partitions whose last slot holds a col-127 pixel: engines
    # cannot address strided partitions, so the patch is a predicated copy
    msk = nc.dram_tensor("msk", [128, COUT], mybir.dt.uint8, kind="ExternalInput").ap()
    out = nc.dram_tensor("out", [P, COUT], f32, kind="ExternalOutput").ap()

    with tile.TileContext(nc) as tc:
        with (
            tc.tile_pool(name="consts", bufs=1) as consts,
            tc.tile_pool(name="xin", bufs=4) as xin_pool,
            tc.tile_pool(name="xt", bufs=4) as xt_pool,
            tc.tile_pool(name="yout", bufs=4) as yout_pool,
            tc.tile_pool(name="tmp", bufs=1) as tmp_pool,
            tc.tile_pool(name="pt", bufs=4, space="PSUM") as pt_pool,
            tc.tile_pool(name="py", bufs=4, space="PSUM") as py_pool,
        ):
            ident = consts.tile([128, 128], f32)
            make_identity(nc, ident)

            # Wl as two K-chunks: w_sb[c, kc, n] = Wl[kc*128 + c, n].
            # fp32r matmul operands must be rounded to fp32r by their
            # producer, so cast during the DMA (SWDGE).
            w_sb = consts.tile([128, 2, COUT], mm_dt)
            w_src = Wl.rearrange("(kc kp) n -> kp kc n", kp=128)
            if mm_dt == f32:
                nc.sync.dma_start(w_sb, w_src)
            else:
                nc.gpsimd.dma_start(w_sb, w_src)

            # bias broadcast to all 128 partitions via ones[128,1] @ bl[1,256]
            ones = consts.tile([1, 128], f32)
            nc.gpsimd.memset(ones, 1.0)
            bl_sb = consts.tile([1, COUT], f32)
            nc.sync.dma_start(bl_sb, bl[None, :])
            bias_ps = py_pool.tile([128, COUT], f32, tag="py")
            nc.tensor.matmul(bias_ps, ones, bl_sb, start=True, stop=True)
            bias_sb = consts.tile([128, COUT], f32)
            nc.scalar.copy(bias_sb, bias_ps)

            msk_sb = consts.tile([128, COUT], mybir.dt.uint8)
            nc.sync.dma_start(msk_sb, msk)

            def conv_group(x_slice, py_out, npart):
                """py_out[0:npart, :] = x_slice @ Wl   (x_slice: [npart, 256])"""
                pt = pt_pool.tile([128, 2, 128], f32, tag="pt")
                nc.tensor.transpose(pt[:, 0, :npart], x_slice[:, 0:128], ident[:npart, :npart])
                nc.tensor.transpose(pt[:, 1, :npart], x_slice[:, 128:256], ident[:npart, :npart])
                xt = xt_pool.tile([128, 2, 128], mm_dt, tag="xt")
                nc.scalar.copy(xt[:, :, :npart], pt[:, :, :npart])
                nc.tensor.matmul(py_out, xt[:, 0, :npart], w_sb[:, 0], start=True, stop=False)
                nc.tensor.matmul(py_out, xt[:, 1, :npart], w_sb[:, 1], start=False, stop=True)

            # ---- last chunk: out [P-CH, P-128) + duplicated final row ----
            O0 = P - CH
            W0 = P - CH + 128  # src window [W0, P)
            NP = (P - W0) // GK  # partitions used
            xbig = xin_pool.tile([128, GK, CIN], f32, tag="xin")
            lsrc = Fl[W0:P].rearrange("(p k) c -> p k c", k=GK)
            lh = GK // 2
            nc.sync.dma_start(xbig[0:NP, 0:2], lsrc[:, 0:2])
            nc.sync.dma_start(xbig[0:NP, 2:lh], lsrc[:, 2:lh])
            nc.sync.dma_start(xbig[0:NP, lh:GK], lsrc[:, lh:GK])
            ybig = yout_pool.tile([128, GK, COUT], f32, tag="yout")
            tmp0 = tmp_pool.tile([128, COUT], f32)
            for k in range(GK):
                py = py_pool.tile([128, COUT], f32, tag="py")
                conv_group(xbig[0:NP, k], py[0:NP], NP)
                if k == 0:
                    # slot target is (p-1, GK-1): shift one partition via DMA
                    nc.vector.tensor_add(tmp0[0:NP], py[0:NP], bias_sb[0:NP])
                else:
                    nc.vector.tensor_add(ybig[0:NP, k - 1], py[0:NP], bias_sb[0:NP])
            nc.sync.dma_start(ybig[0 : NP - 1, GK - 1], tmp0[1:NP])
            nc.vector.copy_predicated(ybig[0:NP, GK - 1], msk_sb[0:NP], ybig[0:NP, GK - 2])
            nc.scalar.dma_start(
                out[O0 : P - 128].rearrange("(p k) c -> p k c", k=GK), ybig[0:NP]
            )
            # final row (n_rows-1) = copy of row n_rows-2 (last 128 slots)
            nrp = 128 // GK
            nc.scalar.dma_start(
                out[P - 128 : P].rearrange("(p k) c -> p k c", k=GK),
                ybig[NP - nrp : NP],
            )

            # ---- uniform chunks: out [CH*c, +CH), src window +129 ----
            for c in range(n_chunks - 1):
                O0 = CH * c
                xbig = xin_pool.tile([128, GK, CIN], f32, tag="xin")
                src_w = Fl[O0 + 129 : O0 + 129 + CH].rearrange("(p k) c -> p k c", k=GK)
                h = GK // 2
                nc.sync.dma_start(xbig[:, 0:h], src_w[:, 0:h])
                nc.sync.dma_start(xbig[:, h:GK], src_w[:, h:GK])
                ybig = yout_pool.tile([128, GK, COUT], f32, tag="yout")
                dst_w = out[O0 : O0 + CH].rearrange("(p k) c -> p k c", k=GK)
                for k in range(GK):
                    py = py_pool.tile([128, COUT], f32, tag="py")
                    conv_group(xbig[:, k], py, 128)
                    nc.vector.tensor_add(ybig[:, k], py, bias_sb)
                    if k == h - 1:
                        nc.scalar.dma_start(dst_w[:, 0:h], ybig[:, 0:h])
                    if GK - 4 > h and k == GK - 5:
                        nc.scalar.dma_start(dst_w[:, h : GK - 4], ybig[:, h : GK - 4])
                # col-127 cells (last slot on masked partitions) duplicate the
                # col-126 value (previous slot): masked predicated copy
                nc.vector.copy_predicated(ybig[:, GK - 1], msk_sb, ybig[:, GK - 2])
                tail0 = max(h, GK - 4)
                nc.scalar.dma_start(dst_w[:, tail0:GK], ybig[:, tail0:GK])

    nc.compile()
    return nc


_cache: dict = {}


def _get_nc():
    if "nc" not in _cache:
        _cache["nc"] = build_nc()
    return _cache["nc"]


def make_mask():
    # partition p's last slot holds pixel GK*p + GK-1; it is a col-127 pixel
    # iff (GK*p + GK-1) % 128 == 127, i.e. p % (128//GK) == 128//GK - 1
    m = np.zeros((128, COUT), dtype=np.uint8)
    step = 128 // GK
    m[step - 1 :: step, :] = 1
    return m


def kernel(Fh, Fl, Wh, bh, Wl, bl):
    nc = _get_nc()
    Fl = np.asarray(Fl, dtype=np.float32)
    Wl_np = np.ascontiguousarray(np.asarray(Wl, dtype=np.float32))
    bl_np = np.ascontiguousarray(np.asarray(bl, dtype=np.float32))
    msk_np = make_mask()
    in_maps = [
        {
            "Fl": np.ascontiguousarray(Fl[b].reshape(H * W, CIN)),
            "Wl": Wl_np,
            "bl": bl_np,
            "msk": msk_np,
        }
        for b in range(B)
    ]
    res = bass_utils.run_bass_kernel_spmd(nc, in_maps, core_ids=list(range(N_CORES)))
    return np.stack(
        [res.results[b]["out"].reshape(H, W, COUT) for b in range(B)], axis=0
    )



# revision 2
# speedup vs baseline: 1.7768x; 1.7768x over previous
"""Trainium2 Bass kernel for nn_L2GESRModule.

Reference computation:
    Fh_conv = Fh @ Wh + bh            (dead: only used via ones_like)
    ESF     = ones_like(Fh_conv)      -> gather indices are a fixed shift
    Y       = Fl @ Wl + bl
    out[b,i,j,:] = Y[b, min(i+1,H-1), min(j+1,W-1), :]

One 1x1-conv GEMM on Fl plus a static (+1,+1) clamped-shift, data-parallel
over batch (1 batch element per core). The Fh/Wh/bh branch is never loaded.

The 2e-2 rel-err budget allows fp16 end-to-end: the host casts Fl/Wl to
fp16 and upcasts the fp16 output, halving HBM traffic (~16.8MB/core ->
~47us at the ~358 GB/s per-core limit). The host also pre-transposes Fl to
[CIN, P] so the kernel needs no PE transposes at all: X^T column slices are
the stationary matmul operand directly.

Flat-pixel layout: image = 16384 pixels; out[O] = Y[O + 129] except col-127
cells (O%128==127) which need Y[O + 128] (clamped col), and the last row
which duplicates row H-2.

Chunks of CH=128*GK pixels. Load xt[p=cin, kc, j, g] = FlT[kc*128+p,
W0 + j*GK + g] (contiguous 4KB per (p,kc) descriptor). Group g's stationary
operand is xt[:, kc, :, g] (column stride GK); psum partition j then holds
pixel W0 + j*GK + g, i.e. ybig[j, g] = Y[W0 + j*GK + g] -- GK *consecutive*
pixels per partition -> GK*0.5 KB contiguous per partition on the store.
Uniform chunk c loads src window [CH*c+129, +CH) so group g writes ybig[:, g]
unshifted. Col-127 cells then duplicate the col-126 value (previous slot, on
partitions p % (128//GK) == 128//GK - 1) via masked copy_predicated. The
last chunk's window would run off the input, so it loads [P-CH+128, P)
(+128-style), shifting group 0's result by one partition via a small
SBUF->SBUF DMA, and duplicates the final row.

Loads go out on the SP HWDGE ring (nc.sync), stores on the ACT HWDGE ring
(nc.scalar) so both physical descriptor rings / all 16 SDMA engines run.
"""

import numpy as np

import concourse.bacc as bacc
import concourse.mybir as mybir
from concourse import bass_utils, tile

B, H, W, CIN, COUT = 8, 128, 128, 256, 256
N_CORES = 8
GK = 16                    # pixel-slots per partition per chunk


def build_nc(n_rows: int = H):
    f16 = mybir.dt.float16
    f32 = mybir.dt.float32
    P = n_rows * W  # total pixels per image
    CH = 128 * GK   # pixels per chunk
    assert P % CH == 0 and P >= CH
    assert 128 % GK == 0
    n_chunks = P // CH

    nc = bacc.Bacc("TRN2", target_bir_lowering=False, debug=False)
    FlT = nc.dram_tensor("FlT", [CIN, P], f16, kind="ExternalInput").ap()
    Wl = nc.dram_tensor("Wl", [CIN, COUT], f16, kind="ExternalInput").ap()
    blb = nc.dram_tensor("blb", [128, COUT], f32, kind="ExternalInput").ap()
    # mask over partitions whose last slot holds a col-127 pixel: engines
    # cannot address strided partitions, so the patch is a predicated copy
    msk = nc.dram_tensor("msk", [128, COUT], mybir.dt.uint8, kind="ExternalInput").ap()
    out = nc.dram_tensor("out", [P, COUT], f16, kind="ExternalOutput").ap()

    with tile.TileContext(nc) as tc:
        with (
            tc.tile_pool(name="consts", bufs=1) as consts,
            tc.tile_pool(name="xt", bufs=4) as xt_pool,
            tc.tile_pool(name="yout", bufs=4) as yout_pool,
            tc.tile_pool(name="tmp", bufs=1) as tmp_pool,
            tc.tile_pool(name="py", bufs=8, space="PSUM") as py_pool,
        ):
            # Wl as two K-chunks: w_sb[c, kc, n] = Wl[kc*128 + c, n]
            w_sb = consts.tile([128, 2, COUT], f16)
            nc.sync.dma_start(w_sb, Wl.rearrange("(kc kp) n -> kp kc n", kp=128))
            bias_sb = consts.tile([128, COUT], f32)
            nc.sync.dma_start(bias_sb, blb)
            msk_sb = consts.tile([128, COUT], mybir.dt.uint8)
            nc.sync.dma_start(msk_sb, msk)

            def conv_group(xt, g, py_out, npj):
                """py_out[0:npj, :] = X[pix, :] @ Wl for pixels W0 + j*GK + g"""
                nc.tensor.matmul(py_out, xt[:, 0, 0:npj, g], w_sb[:, 0],
                                 start=True, stop=False)
                nc.tensor.matmul(py_out, xt[:, 1, 0:npj, g], w_sb[:, 1],
                                 start=False, stop=True)

            # ---- last chunk: out [P-CH, P-128) + duplicated final row ----
            O0 = P - CH
            W0 = P - CH + 128  # src window [W0, P), window shift +128
            NPJ = (P - W0) // GK  # j-partitions used
            xt = xt_pool.tile([128, 2, NPJ, GK], f16, tag="xt")
            lsrc = FlT[:, W0:P].rearrange("(kc p) (j g) -> p kc j g", p=128, g=GK)
            nc.sync.dma_start(xt[:, 0], lsrc[:, 0])
            nc.sync.dma_start(xt[:, 1], lsrc[:, 1])
            ybig = yout_pool.tile([128, GK, COUT], f16, tag="yout")
            tmp0 = tmp_pool.tile([128, COUT], f16)
            for g in range(GK):
                py = py_pool.tile([128, COUT], f32, tag="py")
                conv_group(xt, g, py[0:NPJ], NPJ)
                if g == 0:
                    # slot target is (j-1, GK-1): shift one partition via DMA
                    nc.vector.tensor_add(tmp0[0:NPJ], py[0:NPJ], bias_sb[0:NPJ])
                else:
                    nc.vector.tensor_add(ybig[0:NPJ, g - 1], py[0:NPJ], bias_sb[0:NPJ])
            nc.sync.dma_start(ybig[0 : NPJ - 1, GK - 1], tmp0[1:NPJ])
            nc.vector.copy_predicated(ybig[0:NPJ, GK - 1], msk_sb[0:NPJ], ybig[0:NPJ, GK - 2])
            nc.scalar.dma_start(
                out[O0 : P - 128].rearrange("(p k) c -> p k c", k=GK), ybig[0:NPJ]
            )
            # final row (n_rows-1) = copy of row n_rows-2 (last 128 slots)
            nrp = 128 // GK
            nc.scalar.dma_start(
                out[P - 128 : P].rearrange("(p k) c -> p k c", k=GK),
                ybig[NPJ - nrp : NPJ],
            )

            # ---- uniform chunks: out [CH*c, +CH), src window +129 ----
            for c in range(n_chunks - 1):
                O0 = CH * c
                W0 = O0 + 129
                xt = xt_pool.tile([128, 2, 128, GK], f16, tag="xt")
                src_w = FlT[:, W0 : W0 + CH].rearrange(
                    "(kc p) (j g) -> p kc j g", p=128, g=GK
                )
                nc.sync.dma_start(xt[:, 0], src_w[:, 0])
                nc.sync.dma_start(xt[:, 1], src_w[:, 1])
                ybig = yout_pool.tile([128, GK, COUT], f16, tag="yout")
                dst_w = out[O0 : O0 + CH].rearrange("(p k) c -> p k c", k=GK)
                h = GK // 2
                for g in range(GK):
                    py = py_pool.tile([128, COUT], f32, tag="py")
                    conv_group(xt, g, py, 128)
                    nc.vector.tensor_add(ybig[:, g], py, bias_sb)
                    if g == h - 1:
                        nc.scalar.dma_start(dst_w[:, 0:h], ybig[:, 0:h])
                    if GK - 4 > h and g == GK - 5:
                        nc.scalar.dma_start(dst_w[:, h : GK - 4], ybig[:, h : GK - 4])
                # col-127 cells (last slot on masked partitions) duplicate the
                # col-126 value (previous slot): masked predicated copy
                nc.vector.copy_predicated(ybig[:, GK - 1], msk_sb, ybig[:, GK - 2])
                tail0 = max(h, GK - 4)
                nc.scalar.dma_start(dst_w[:, tail0:GK], ybig[:, tail0:GK])

    nc.compile()
    return nc


_cache: dict = {}


def _get_nc():
    if "nc" not in _cache:
        _cache["nc"] = build_nc()
    return _cache["nc"]


def make_mask():
    # partition j's last slot holds pixel GK*j + GK-1; it is a col-127 pixel
    # iff (GK*j + GK-1) % 128 == 127, i.e. j % (128//GK) == 128//GK - 1
    m = np.zeros((128, COUT), dtype=np.uint8)
    step = 128 // GK
    m[step - 1 :: step, :] = 1
    return m


def make_in_maps(Fl, Wl, bl):
    Fl = np.asarray(Fl, dtype=np.float32)
    Wl16 = np.ascontiguousarray(np.asarray(Wl).astype(np.float16))
    blb = np.ascontiguousarray(
        np.broadcast_to(np.asarray(bl, dtype=np.float32), (128, COUT))
    )
    msk_np = make_mask()
    return [
        {
            "FlT": np.ascontiguousarray(
                Fl[b].reshape(H * W, CIN).T, dtype=np.float16
            ),
            "Wl": Wl16,
            "blb": blb,
            "msk": msk_np,
        }
        for b in range(B)
    ]


def kernel(Fh, Fl, Wh, bh, Wl, bl):
    nc = _get_nc()
    in_maps = make_in_maps(Fl, Wl, bl)
    res = bass_utils.run_bass_kernel_spmd(nc, in_maps, core_ids=list(range(N_CORES)))
    return np.stack(
        [
            res.results[b]["out"].astype(np.float32).reshape(H, W, COUT)
            for b in range(B)
        ],
        axis=0,
    )
